# revision 1
# baseline (speedup 1.0000x reference)
"""Trainium2 Bass kernel for InvariantMessage GNN message passing.

out[e, :] = (MLP(s_j)[nbrs[e,1]]) * ((rbf(dist[e]) @ W_rbf + b_rbf) * env(dist[e]))

Strategy (8 cores, edge-parallel):
- Each core redundantly computes inv = MLP(s_j) for all nodes into two DRAM
  tables (invA: nodes < 32768, invB: rest).
- Edges are sharded 100k/core; the host pre-partitions each shard into
  low/high node-index groups (gathered from invA/invB), pads each group to a
  uniform number of 3072-edge chunks across cores (pad slots gather row 0 and
  are discarded), and precomputes broadcast/transposed dist layouts. Gathers
  use indirect_dma_start, 128 rows (one per partition) per instruction --
  the HW-validated semantics (dma_gather int16 hung the device under Tile).
- Per 512 edges: sin args n*pi*d/5 are range-reduced to [-pi, pi] with the
  fp32 magic-number rounding trick, evaluated by ScalarE Sin in a 4x(32-row)
  quadrant-packed layout [sin rows 0..19 + raw d row 20 per group], then a
  K=21 matmul against [W_rbf; b_rbf] gives w*d in PSUM; scaling by env/d and
  the gathered phi finishes the edge.
"""
import sys

sys.path.insert(0, "/opt/trn_rl_repo")

import numpy as np

import concourse.tile as tile
from concourse import bass, bacc, mybir
from concourse.bass_utils import run_bass_kernel_spmd

F32 = mybir.dt.float32
I16 = mybir.dt.int16

N_CORES = 8
N_ATOMS = 50000
N_EDGES = 800000
D = 128
NB = 20
CUTOFF = 5.0
MAGIC = float(np.float32(1.5 * 2**23))

SPLIT = 32768                     # invA rows; invB = rest
E_CORE = N_EDGES // N_CORES       # 100000
GCH = 3072                        # edges per dma_gather / out-dma chunk
NODE_PAD = 50176                  # 98 * 512
NCH_NODE = NODE_PAD // 512        # 98
VB = NODE_PAD - SPLIT             # 17408 invB rows (34*512)
SPLIT_CHUNK = SPLIT // 512        # 64


def build_nc(n_gchunks, low_chunks):
    """Build the Bass program: chunk g gathers from invA if g < low_chunks
    else invB; every chunk is full (GCH valid indices)."""
    nc = bacc.Bacc(None, target_bir_lowering=False)
    E_pad = n_gchunks * GCH

    s_jT = nc.dram_tensor("s_jT", [D, NODE_PAD], F32, kind="ExternalInput")
    W1 = nc.dram_tensor("W1", [D, D], F32, kind="ExternalInput")
    W2 = nc.dram_tensor("W2", [D, D], F32, kind="ExternalInput")
    b1c = nc.dram_tensor("b1c", [D, 1], F32, kind="ExternalInput")
    b2c = nc.dram_tensor("b2c", [D, 1], F32, kind="ExternalInput")
    wext = nc.dram_tensor("wext", [D, D], F32, kind="ExternalInput")
    coef2 = nc.dram_tensor("coef2", [D, 1], F32, kind="ExternalInput")
    ident = nc.dram_tensor("ident", [D, D], F32, kind="ExternalInput")
    nhp = nc.dram_tensor("nhp", [D, 1], F32, kind="ExternalInput")
    dist_b = nc.dram_tensor("dist_b", [(E_pad // 384) * D, D], F32,
                            kind="ExternalInput")
    dist_t2 = nc.dram_tensor("dist_t2", [(E_pad // 1536) * D, 12], F32,
                             kind="ExternalInput")
    idx32 = nc.dram_tensor("idx32", [n_gchunks * GCH, 1], mybir.dt.int32,
                           kind="ExternalInput")
    out_dev = nc.dram_tensor("out_dev", [E_pad, D], F32, kind="ExternalOutput")

    invA = nc.dram_tensor("invA", [SPLIT, D], F32, kind="ExternalInput")
    invB = nc.dram_tensor("invB", [VB, D], F32, kind="ExternalInput")

    with tile.TileContext(nc) as tc:
        with tc.tile_pool(name="const", bufs=1) as cpool, \
             tc.tile_pool(name="mlp", bufs=3) as mpool, \
             tc.tile_pool(name="mlppsum", bufs=1, space="PSUM") as mpsum, \
             tc.tile_pool(name="tpsum", bufs=2, space="PSUM") as tpsum, \
             tc.tile_pool(name="edge", bufs=3) as epool, \
             tc.tile_pool(name="big", bufs=2) as bpool, \
             tc.tile_pool(name="wpsum", bufs=4, space="PSUM") as wpsum:

            w1_sb = cpool.tile([D, D], F32)
            nc.sync.dma_start(out=w1_sb[:], in_=W1[:])
            w2_sb = cpool.tile([D, D], F32)
            nc.sync.dma_start(out=w2_sb[:], in_=W2[:])
            b1_sb = cpool.tile([D, 1], F32)
            nc.sync.dma_start(out=b1_sb[:], in_=b1c[:])
            b2_sb = cpool.tile([D, 1], F32)
            nc.sync.dma_start(out=b2_sb[:], in_=b2c[:])
            wext_sb = cpool.tile([D, D], F32)
            nc.sync.dma_start(out=wext_sb[:], in_=wext[:])
            coef_sb = cpool.tile([D, 1], F32)
            nc.sync.dma_start(out=coef_sb[:], in_=coef2[:])
            id_sb = cpool.tile([D, D], F32)
            nc.sync.dma_start(out=id_sb[:], in_=ident[:])
            nhp_sb = cpool.tile([D, 1], F32)
            nc.sync.dma_start(out=nhp_sb[:], in_=nhp[:])

            # ---- Phase 1: node MLP -> invA / invB ----
            for i in range(NCH_NODE):
                s_t = mpool.tile([D, 512], F32, tag="s")
                nc.sync.dma_start(out=s_t[:], in_=s_jT[:, i * 512:(i + 1) * 512])
                ph = mpsum.tile([D, 512], F32, tag="ph")
                nc.tensor.matmul(out=ph[:], lhsT=w1_sb[:], rhs=s_t[:],
                                 start=True, stop=True)
                h_t = mpool.tile([D, 512], F32, tag="h")
                nc.scalar.activation(out=h_t[:], in_=ph[:],
                                     func=mybir.ActivationFunctionType.Silu,
                                     bias=b1_sb[:, 0:1], scale=1.0)
                pi = mpsum.tile([D, 512], F32, tag="pi")
                nc.tensor.matmul(out=pi[:], lhsT=w2_sb[:], rhs=h_t[:],
                                 start=True, stop=True)
                iv = mpool.tile([D, 512], F32, tag="iv")
                nc.vector.tensor_scalar_add(out=iv[:], in0=pi[:],
                                            scalar1=b2_sb[:, 0:1])
                for j in range(4):
                    pt = tpsum.tile([D, D], F32, tag="pt")
                    nc.tensor.transpose(out=pt[:], in_=iv[:, j * D:(j + 1) * D],
                                        identity=id_sb[:])
                    ot = mpool.tile([D, D], F32, tag="ot")
                    nc.scalar.copy(out=ot[:], in_=pt[:])
                    n0 = i * 512 + j * D
                    if i < SPLIT_CHUNK:
                        nc.sync.dma_start(out=invA[n0:n0 + D, :], in_=ot[:])
                    else:
                        m0 = n0 - SPLIT
                        nc.sync.dma_start(out=invB[m0:m0 + D, :], in_=ot[:])

            # ---- Phase 2: edges ----
            for g in range(n_gchunks):
                ix = epool.tile([D, GCH // D], mybir.dt.int32, tag="ix")
                nc.sync.dma_start(
                    out=ix[:],
                    in_=idx32[g * GCH:(g + 1) * GCH, :].rearrange(
                        "(s p) o -> p (s o)", p=D))
                phi = bpool.tile([D, GCH // D, D], F32, tag="phi")
                table = invA if g < low_chunks else invB
                for s in range(GCH // D):
                    nc.gpsimd.indirect_dma_start(
                        out=phi[:, s, :], out_offset=None, in_=table[:],
                        in_offset=bass.IndirectOffsetOnAxis(
                            ap=ix[:, s:s + 1], axis=0))
                out_sb = bpool.tile([D, GCH // D, D], F32, tag="osb")
                for c2 in range(GCH // 1536):
                    e0 = g * GCH + c2 * 1536
                    cb0 = e0 // 384
                    db = epool.tile([D, 512], F32, tag="db")
                    for k in range(4):
                        nc.sync.dma_start(
                            out=db[:, k * D:(k + 1) * D],
                            in_=dist_b[(cb0 + k) * D:(cb0 + k + 1) * D, :])
                    dt = epool.tile([D, 12], F32, tag="dt")
                    c2g = e0 // 1536
                    nc.sync.dma_start(out=dt[:],
                                      in_=dist_t2[c2g * D:(c2g + 1) * D, :])
                    u = epool.tile([D, 512], F32, tag="u")
                    nc.scalar.activation(out=u[:], in_=db[:],
                                         func=mybir.ActivationFunctionType.Copy,
                                         scale=coef_sb[:, 0:1])
                    kf = epool.tile([D, 512], F32, tag="kf")
                    nc.vector.tensor_scalar(out=kf[:], in0=u[:],
                                            scalar1=MAGIC, scalar2=MAGIC,
                                            op0=mybir.AluOpType.add,
                                            op1=mybir.AluOpType.subtract)
                    v = epool.tile([D, 512], F32, tag="v")
                    nc.vector.tensor_tensor(out=v[:], in0=u[:], in1=kf[:],
                                            op=mybir.AluOpType.subtract)
                    for j in range(3):
                        nc.scalar.activation(
                            out=db[32 * j:32 * j + NB, :],
                            in_=v[32 * j:32 * j + NB, :],
                            func=mybir.ActivationFunctionType.Sin,
                            scale=float(2 * np.pi))
                    rd = epool.tile([D, 12], F32, tag="rd")
                    nc.vector.reciprocal(out=rd[:], in_=dt[:])
                    cs = epool.tile([D, 12], F32, tag="cs")
                    nc.scalar.activation(out=cs[:], in_=dt[:],
                                         func=mybir.ActivationFunctionType.Sin,
                                         scale=float(np.pi / CUTOFF),
                                         bias=nhp_sb[:, 0:1])
                    env = epool.tile([D, 12], F32, tag="env")
                    nc.vector.tensor_scalar(out=env[:], in0=cs[:],
                                            scalar1=-0.5, scalar2=0.5,
                                            op0=mybir.AluOpType.mult,
                                            op1=mybir.AluOpType.add)
                    scl = epool.tile([D, 12], F32, tag="scl")
                    nc.vector.tensor_tensor(out=scl[:], in0=env[:], in1=rd[:],
                                            op=mybir.AluOpType.mult)
                    for t in range(12):
                        k, j = t // 3, t % 3
                        pw = wpsum.tile([D, D], F32, tag="pw")
                        nc.tensor.matmul(
                            out=pw[:],
                            lhsT=db[32 * j:32 * j + NB + 1, k * D:(k + 1) * D],
                            rhs=wext_sb[32 * j:32 * j + NB + 1, :],
                            start=True, stop=True)
                        ws = epool.tile([D, D], F32, tag="ws")
                        nc.scalar.activation(
                            out=ws[:], in_=pw[:],
                            func=mybir.ActivationFunctionType.Copy,
                            scale=scl[:, t:t + 1])
                        slot = c2 * 12 + t
                        nc.vector.tensor_tensor(
                            out=out_sb[:, slot, :], in0=ws[:],
                            in1=phi[:, slot, :], op=mybir.AluOpType.mult)
                nc.sync.dma_start(
                    out=out_dev[g * GCH:(g + 1) * GCH, :].rearrange(
                        "(s p) f -> p s f", p=D),
                    in_=out_sb[:])
    nc.finalize()
    return nc


_NC_CACHE = {}


def kernel(s_j, dist, nbrs, W1, b1, W2, b2, W_rbf, b_rbf):
    s_j = np.asarray(s_j, dtype=np.float32)
    dist = np.asarray(dist, dtype=np.float32)
    idx_all = np.asarray(nbrs)[:, 1].astype(np.int32)

    s_jT = np.zeros((D, NODE_PAD), dtype=np.float32)
    s_jT[:, :N_ATOMS] = s_j.T
    w21 = np.concatenate([np.asarray(W_rbf, np.float32),
                          np.asarray(b_rbf, np.float32)[None, :]], axis=0)
    wext = np.zeros((D, D), dtype=np.float32)
    for qj in range(3):
        wext[32 * qj:32 * qj + NB + 1] = w21
    coef2 = np.zeros((D, 1), dtype=np.float32)
    for p in range(96):
        n = p % 32
        if n < NB:
            coef2[p, 0] = (n + 1) / 10.0
    common = {
        "s_jT": s_jT,
        "W1": np.asarray(W1, np.float32),
        "W2": np.asarray(W2, np.float32),
        "b1c": np.asarray(b1, np.float32).reshape(D, 1),
        "b2c": np.asarray(b2, np.float32).reshape(D, 1),
        "wext": wext,
        "coef2": coef2,
        "ident": np.eye(D, dtype=np.float32),
        "nhp": np.full((D, 1), -np.pi / 2, dtype=np.float32),
        "invA": np.zeros((SPLIT, D), dtype=np.float32),
        "invB": np.zeros((VB, D), dtype=np.float32),
    }

    # shard stats -> uniform chunk counts across cores
    shards = []
    for c in range(N_CORES):
        sl = slice(c * E_CORE, (c + 1) * E_CORE)
        ish, dsh = idx_all[sl], dist[sl]
        low = ish < SPLIT
        shards.append((ish, dsh, low, int(low.sum())))
    max_low = max(s[3] for s in shards)
    max_high = max(E_CORE - s[3] for s in shards)
    LC = (max_low + GCH - 1) // GCH        # low chunks
    HC = (max_high + GCH - 1) // GCH       # high chunks
    n_g = LC + HC
    LP = LC * GCH
    E_pad = n_g * GCH

    in_maps, metas = [], []
    for c in range(N_CORES):
        ish, dsh, low, n_low = shards[c]
        n_high = E_CORE - n_low
        perm = np.argsort(~low, kind="stable")
        idx_pad = np.zeros(E_pad, dtype=np.int32)
        dist_pad = np.ones(E_pad, dtype=np.float32)
        idx_p, dist_p = ish[perm], dsh[perm]
        idx_pad[:n_low] = idx_p[:n_low]
        dist_pad[:n_low] = dist_p[:n_low]
        idx_pad[LP:LP + n_high] = idx_p[n_low:] - SPLIT
        dist_pad[LP:LP + n_high] = dist_p[n_low:]
        # dist_b[c*128+p, e] = dist_pad[c*384 + (p//32)*128 + e], p<96
        dseg = dist_pad.reshape(-1, 3, 128)
        db3 = np.repeat(dseg, 32, axis=1)
        db = np.concatenate(
            [db3, np.ones((db3.shape[0], 32, 128), np.float32)],
            axis=1).reshape(-1, 128)
        # dist_t2[c2*128+p, t] = dist_pad[c2*1536 + t*128 + p]
        dt2 = np.ascontiguousarray(
            dist_pad.reshape(-1, 12, 128).transpose(0, 2, 1).reshape(-1, 12))
        metas.append((perm, n_low, n_high))
        in_maps.append(dict(common, dist_b=db, dist_t2=dt2,
                            idx32=idx_pad.reshape(-1, 1)))

    ckey = (n_g, LC)
    if ckey not in _NC_CACHE:
        _NC_CACHE[ckey] = build_nc(n_g, LC)
    nc = _NC_CACHE[ckey]

    res = run_bass_kernel_spmd(nc, in_maps, list(range(N_CORES)))
    out = np.empty((N_EDGES, D), dtype=np.float32)
    for c in range(N_CORES):
        perm, n_low, n_high = metas[c]
        od = res.results[c]["out_dev"]
        shard = np.empty((E_CORE, D), dtype=np.float32)
        shard[perm] = np.concatenate([od[:n_low], od[LP:LP + n_high]], axis=0)
        out[c * E_CORE:(c + 1) * E_CORE] = shard
    return out



# revision 2
# speedup vs baseline: 1.0676x; 1.0676x over previous
"""Trainium2 Bass kernel for InvariantMessage GNN message passing.

out[e, :] = (MLP(s_j)[nbrs[e,1]]) * ((rbf(dist[e]) @ W_rbf + b_rbf) * env(dist[e]))

The axon tunnel (~60-100 MB/s up, ~30-50 MB/s down) dominates wall time —
measured device execution is ~0.1 s while the baseline call took ~28 s — so
this version minimizes bytes on the wire rather than device cycles:

  - fp16 everywhere on device (weights, node features, inv table, rbf
    matmul). HW-validated: fp16 matmul is exact, and a single 50176-row
    fp16 table supports indirect-DMA gathers with int32 row indices up to
    50175 (the old invA/invB 32768-split was only needed for 512B f32 rows).
  - the inv table is Internal DRAM scratch - nothing uploaded for it.
  - node features are sharded 8-ways (1.6 MB/core fp16); each core runs the
    MLP on its 6272-node slice and an on-device HBM AllGather (replica
    group [0..7]) assembles the full 50176-row table on every core.
  - dist/idx are uploaded raw per edge shard (0.4 MB each per core) in a
    host-pretransposed [NCH, 128, 24] layout so all device DMAs are
    contiguous; the [21, e] rbf lhsT is built on device: sin in an
    edge-partition layout [128e, 20] via fp32 magic-number range reduction,
    pre-scaled by env(d)/d, then one TensorE transpose per 128 edges.
  - output is int8 with a per-edge fp32 scale (f32->int8 cast is
    round-to-nearest with saturation on HW); the host does a single-pass
    strided dequant-multiply into the final array. 13.4 MB/core down
    instead of 53.5 MB/core.

Per-edge math on device (col = 128 edges):
  u = coef_k * d            (coef_k = (k+1)/10, i.e. k pi d / 5 / 2pi)
  v = u - round(u)          (fp32 magic-number rounding)
  sv = [sin(2 pi v) k<20 ; d] * (env(d)/d)      # [128e, 21] fp16
  lhsT = sv^T via TensorE transpose             # [21, 128e]
  ws = lhsT^T @ [W_rbf; b_rbf]                  # PSUM f32 [128e, 128f]
  m = ws * phi_gathered                         # f32
  q = int8(m * 127/absmax_row), scale_out = absmax_row/127

Edges are sharded 100000/core, padded to 33 chunks of 3072 (pad slots gather
row 0 with d=1 and are dropped on the host).
"""
import sys

sys.path.insert(0, "/opt/trn_rl_repo")

import numpy as np

import concourse.tile as tile
from concourse import bass, bacc, mybir
from concourse.bass_utils import run_bass_kernel_spmd

F32 = mybir.dt.float32
F16 = mybir.dt.float16
I32 = mybir.dt.int32
I8 = mybir.dt.int8

N_CORES = 8
N_ATOMS = 50000
N_EDGES = 800000
D = 128
NB = 20
CUTOFF = 5.0
MAGIC = float(np.float32(1.5 * 2**23))

NODE_PAD = 50176                  # 98 * 512
NSH = NODE_PAD // N_CORES         # 6272 nodes per core (12×512 + 128)
NODE_CHUNKS = [512] * 12 + [128]  # column chunking of the per-core slice
E_CORE = N_EDGES // N_CORES       # 100000
GCH = 3072                        # edges per chunk
SC = GCH // D                     # 24 cols of 128 edges per chunk
NCH = (E_CORE + GCH - 1) // GCH   # 33
E_PAD = NCH * GCH                 # 101376


def build_nc():
    nc = bacc.Bacc(None, target_bir_lowering=False)

    s_jT = nc.dram_tensor("s_jT", [D, NSH], F16, kind="ExternalInput")
    W1h = nc.dram_tensor("W1h", [D, D], F16, kind="ExternalInput")
    W2h = nc.dram_tensor("W2h", [D, D], F16, kind="ExternalInput")
    b1c = nc.dram_tensor("b1c", [D, 1], F32, kind="ExternalInput")
    b2c = nc.dram_tensor("b2c", [D, 1], F32, kind="ExternalInput")
    wextb = nc.dram_tensor("wextb", [32, D], F16, kind="ExternalInput")
    identh = nc.dram_tensor("identh", [D, D], F16, kind="ExternalInput")
    coefC = nc.dram_tensor("coefC", [D, NB], F32, kind="ExternalInput")
    nhp = nc.dram_tensor("nhp", [D, 1], F32, kind="ExternalInput")
    distL = nc.dram_tensor("distL", [NCH, D, SC], F32, kind="ExternalInput")
    idxL = nc.dram_tensor("idxL", [NCH, D, SC], I32, kind="ExternalInput")
    qout = nc.dram_tensor("qout", [NCH, D, SC, D], I8, kind="ExternalOutput")
    sout = nc.dram_tensor("sout", [NCH, D, SC], F32, kind="ExternalOutput")

    invS = nc.dram_tensor("invS", [NSH, D], F16)       # this core's inv slice
    invT = nc.dram_tensor("invT", [NODE_PAD, D], F16)  # AllGather of invS

    with tile.TileContext(nc) as tc:
        with tc.tile_pool(name="const", bufs=1) as cpool, \
             tc.tile_pool(name="mlp", bufs=3) as mpool, \
             tc.tile_pool(name="mlppsum", bufs=1, space="PSUM") as mpsum, \
             tc.tile_pool(name="tpsum", bufs=2, space="PSUM") as tpsum, \
             tc.tile_pool(name="edge", bufs=3) as epool, \
             tc.tile_pool(name="big", bufs=2) as bpool, \
             tc.tile_pool(name="wpsum", bufs=2, space="PSUM") as wpsum:

            w1_sb = cpool.tile([D, D], F16)
            nc.sync.dma_start(out=w1_sb[:], in_=W1h[:])
            w2_sb = cpool.tile([D, D], F16)
            nc.sync.dma_start(out=w2_sb[:], in_=W2h[:])
            b1_sb = cpool.tile([D, 1], F32)
            nc.sync.dma_start(out=b1_sb[:], in_=b1c[:])
            b2_sb = cpool.tile([D, 1], F32)
            nc.sync.dma_start(out=b2_sb[:], in_=b2c[:])
            wext_sb = cpool.tile([32, D], F16)
            nc.sync.dma_start(out=wext_sb[:], in_=wextb[:])
            id_sb = cpool.tile([D, D], F16)
            nc.sync.dma_start(out=id_sb[:], in_=identh[:])
            coef_sb = cpool.tile([D, NB], F32)
            nc.sync.dma_start(out=coef_sb[:], in_=coefC[:])
            nhp_sb = cpool.tile([D, 1], F32)
            nc.sync.dma_start(out=nhp_sb[:], in_=nhp[:])

            # ---- Phase 1: node MLP for this core's 6272-node slice ----
            n0 = 0
            for ncols in NODE_CHUNKS:
                s_t = mpool.tile([D, 512], F16, tag="s")
                nc.sync.dma_start(out=s_t[:, 0:ncols],
                                  in_=s_jT[:, n0:n0 + ncols])
                ph = mpsum.tile([D, 512], F32, tag="ph")
                nc.tensor.matmul(out=ph[:, 0:ncols], lhsT=w1_sb[:],
                                 rhs=s_t[:, 0:ncols], start=True, stop=True)
                h_t = mpool.tile([D, 512], F16, tag="h")
                nc.scalar.activation(out=h_t[:, 0:ncols], in_=ph[:, 0:ncols],
                                     func=mybir.ActivationFunctionType.Silu,
                                     bias=b1_sb[:, 0:1], scale=1.0)
                pi = mpsum.tile([D, 512], F32, tag="pi")
                nc.tensor.matmul(out=pi[:, 0:ncols], lhsT=w2_sb[:],
                                 rhs=h_t[:, 0:ncols], start=True, stop=True)
                iv = mpool.tile([D, 512], F16, tag="iv")
                nc.vector.tensor_scalar_add(out=iv[:, 0:ncols],
                                            in0=pi[:, 0:ncols],
                                            scalar1=b2_sb[:, 0:1])
                for j in range(ncols // D):
                    pt = tpsum.tile([D, D], F16, tag="pt")
                    nc.tensor.transpose(out=pt[:], in_=iv[:, j * D:(j + 1) * D],
                                        identity=id_sb[:])
                    ot = mpool.tile([D, D], F16, tag="ot")
                    nc.scalar.copy(out=ot[:], in_=pt[:])
                    m0 = n0 + j * D
                    nc.sync.dma_start(out=invS[m0:m0 + D, :], in_=ot[:])
                n0 += ncols

            # ---- AllGather the 8 slices into the full inv table ----
            nc.gpsimd.collective_compute(
                "AllGather", mybir.AluOpType.bypass,
                replica_groups=[list(range(N_CORES))],
                ins=[invS[:, :]], outs=[invT[:, :]])

            # ---- Phase 2: edges ----
            for g in range(NCH):
                ix = epool.tile([D, SC], I32, tag="ix")
                nc.sync.dma_start(out=ix[:], in_=idxL[g])
                dt = epool.tile([D, SC], F32, tag="dt")
                nc.sync.dma_start(out=dt[:], in_=distL[g])

                rd = epool.tile([D, SC], F32, tag="rd")
                nc.vector.reciprocal(out=rd[:], in_=dt[:])
                cs = epool.tile([D, SC], F32, tag="cs")
                nc.scalar.activation(out=cs[:], in_=dt[:],
                                     func=mybir.ActivationFunctionType.Sin,
                                     scale=float(np.pi / CUTOFF),
                                     bias=nhp_sb[:, 0:1])
                env = epool.tile([D, SC], F32, tag="env")
                nc.vector.tensor_scalar(out=env[:], in0=cs[:],
                                        scalar1=-0.5, scalar2=0.5,
                                        op0=mybir.AluOpType.mult,
                                        op1=mybir.AluOpType.add)
                scl = epool.tile([D, SC], F32, tag="scl")
                nc.vector.tensor_tensor(out=scl[:], in0=env[:], in1=rd[:],
                                        op=mybir.AluOpType.mult)

                phig = bpool.tile([D, SC, D], F16, tag="phi")
                for s in range(SC):
                    nc.gpsimd.indirect_dma_start(
                        out=phig[:, s, :], out_offset=None, in_=invT[:],
                        in_offset=bass.IndirectOffsetOnAxis(
                            ap=ix[:, s:s + 1], axis=0))

                msb = bpool.tile([D, SC, D], F32, tag="msb")
                amax = epool.tile([D, SC], F32, tag="amax")
                for s in range(SC):
                    u = epool.tile([D, NB], F32, tag="u")
                    nc.scalar.activation(out=u[:], in_=coef_sb[:],
                                         func=mybir.ActivationFunctionType.Copy,
                                         scale=dt[:, s:s + 1])
                    kf = epool.tile([D, NB], F32, tag="kf")
                    nc.vector.tensor_scalar(out=kf[:], in0=u[:],
                                            scalar1=MAGIC, scalar2=MAGIC,
                                            op0=mybir.AluOpType.add,
                                            op1=mybir.AluOpType.subtract)
                    v = epool.tile([D, NB], F32, tag="v")
                    nc.vector.tensor_tensor(out=v[:], in0=u[:], in1=kf[:],
                                            op=mybir.AluOpType.subtract)
                    sv = epool.tile([D, NB + 1], F16, tag="sv")
                    nc.scalar.activation(out=sv[:, 0:NB], in_=v[:],
                                         func=mybir.ActivationFunctionType.Sin,
                                         scale=float(2 * np.pi))
                    nc.scalar.copy(out=sv[:, NB:NB + 1], in_=dt[:, s:s + 1])
                    svs = epool.tile([D, NB + 1], F16, tag="svs")
                    nc.vector.tensor_scalar_mul(out=svs[:], in0=sv[:],
                                                scalar1=scl[:, s:s + 1])
                    pt2 = tpsum.tile([32, D], F16, tag="pt2")
                    nc.tensor.transpose(out=pt2[0:NB + 1, :], in_=svs[:],
                                        identity=id_sb[:])
                    lt = epool.tile([32, D], F16, tag="lt")
                    nc.scalar.copy(out=lt[0:NB + 1, :], in_=pt2[0:NB + 1, :])
                    pw = wpsum.tile([D, D], F32, tag="pw")
                    nc.tensor.matmul(out=pw[:], lhsT=lt[0:NB + 1, :],
                                     rhs=wext_sb[0:NB + 1, :],
                                     start=True, stop=True)
                    nc.vector.tensor_tensor(out=msb[:, s, :], in0=pw[:],
                                            in1=phig[:, s, :],
                                            op=mybir.AluOpType.mult)
                    nc.vector.tensor_reduce(out=amax[:, s:s + 1],
                                            in_=msb[:, s, :],
                                            axis=mybir.AxisListType.X,
                                            op=mybir.AluOpType.max,
                                            apply_absolute_value=True)

                amc = epool.tile([D, SC], F32, tag="amc")
                nc.vector.tensor_scalar_max(out=amc[:], in0=amax[:],
                                            scalar1=1e-20)
                sct = epool.tile([D, SC], F32, tag="sct")
                nc.vector.tensor_scalar_mul(out=sct[:], in0=amc[:],
                                            scalar1=float(1.0 / 127.0))
                nc.sync.dma_start(out=sout[g], in_=sct[:])
                rst = epool.tile([D, SC], F32, tag="rst")
                nc.vector.reciprocal(out=rst[:], in_=sct[:])

                qsb = bpool.tile([D, SC, D], I8, tag="qsb")
                for s in range(SC):
                    nc.scalar.activation(out=qsb[:, s, :], in_=msb[:, s, :],
                                         func=mybir.ActivationFunctionType.Copy,
                                         scale=rst[:, s:s + 1])
                nc.sync.dma_start(out=qout[g], in_=qsb[:])
    nc.finalize()
    return nc


_NC_CACHE = {}


def kernel(s_j, dist, nbrs, W1, b1, W2, b2, W_rbf, b_rbf):
    s_j = np.asarray(s_j, dtype=np.float32)
    dist = np.asarray(dist, dtype=np.float32)
    idx_all = np.asarray(nbrs)[:, 1].astype(np.int32)

    s_jT_full = np.zeros((D, NODE_PAD), dtype=np.float16)
    s_jT_full[:, :N_ATOMS] = s_j.T
    wextb = np.zeros((32, D), dtype=np.float16)
    wextb[:NB] = np.asarray(W_rbf, np.float32)
    wextb[NB] = np.asarray(b_rbf, np.float32)
    coefC = np.broadcast_to(
        (np.arange(1, NB + 1, dtype=np.float32) / 10.0)[None, :],
        (D, NB)).copy()
    common = {
        "W1h": np.asarray(W1, np.float32).astype(np.float16),
        "W2h": np.asarray(W2, np.float32).astype(np.float16),
        "b1c": np.asarray(b1, np.float32).reshape(D, 1),
        "b2c": np.asarray(b2, np.float32).reshape(D, 1),
        "wextb": wextb,
        "identh": np.eye(D, dtype=np.float16),
        "coefC": coefC,
        "nhp": np.full((D, 1), -np.pi / 2, dtype=np.float32),
    }

    in_maps = []
    for c in range(N_CORES):
        sl = slice(c * E_CORE, (c + 1) * E_CORE)
        idx_pad = np.zeros(E_PAD, dtype=np.int32)
        idx_pad[:E_CORE] = idx_all[sl]
        dist_pad = np.ones(E_PAD, dtype=np.float32)
        dist_pad[:E_CORE] = dist[sl]
        idxL = np.ascontiguousarray(
            idx_pad.reshape(NCH, SC, D).transpose(0, 2, 1))
        distL = np.ascontiguousarray(
            dist_pad.reshape(NCH, SC, D).transpose(0, 2, 1))
        s_jT = np.ascontiguousarray(s_jT_full[:, c * NSH:(c + 1) * NSH])
        in_maps.append(dict(common, s_jT=s_jT, distL=distL, idxL=idxL))

    if "nc" not in _NC_CACHE:
        _NC_CACHE["nc"] = build_nc()
    nc = _NC_CACHE["nc"]

    res = run_bass_kernel_spmd(nc, in_maps, list(range(N_CORES)))
    out = np.empty((N_EDGES, D), dtype=np.float32)
    nfull = E_CORE // GCH                     # 32 full chunks per core
    rem = E_CORE - nfull * GCH                # 1696 edges in the tail chunk
    rs = rem // D                             # 13 full cols
    r2 = rem - rs * D                         # 32 edges in the last col
    for c in range(N_CORES):
        q = res.results[c]["qout"]            # [NCH, D, SC, D] int8
        sc = res.results[c]["sout"]           # [NCH, D, SC] f32
        o = out[c * E_CORE:(c + 1) * E_CORE]
        # single-pass dequant straight into the output (edge-major view)
        np.multiply(q[:nfull].transpose(0, 2, 1, 3),
                    sc[:nfull].transpose(0, 2, 1)[..., None],
                    out=o[:nfull * GCH].reshape(nfull, SC, D, D))
        qt = q[nfull].transpose(1, 0, 2)      # [SC, D, D]
        st = sc[nfull].T                      # [SC, D]
        ot = o[nfull * GCH:]
        np.multiply(qt[:rs], st[:rs, :, None],
                    out=ot[:rs * D].reshape(rs, D, D))
        if r2:
            np.multiply(qt[rs, :r2], st[rs, :r2, None], out=ot[rs * D:])
    return out
